# revision 2
# baseline (speedup 1.0000x reference)
"""MPI compositing + homography warp kernel for Trainium2 (8 NeuronCores).

For each of P=32 fronto-parallel planes and S=4 source images: composite
per-plane channels (net transmittance T, accumulated-over acc, full-over
bro, source image src -> 10 channels), then bilinear-warp each (plane, src)
channel stack by a plane/source-dependent homography. Output (P, S, 10, H, W).

Structure exploited: the target->source homography here has identity
rotation and shared intrinsics, so sample coordinate ix depends only on x
and iy only on y.  The bilinear gather (zero padding) then factorizes
EXACTLY into two small banded matrices applied left/right:

    warped = Wy @ S @ Wx^T        per (plane, src, channel)

with per-tap validity folded into the weights.  Wy/Wx are built on the host
from the pose inputs and executed as PE matmuls (fp32r = full-rate fp32 at
N>=256).

Sharding: core = (s, h) in 4 sources x 2 output-row-halves.  The cross-plane
compositing scan is pointwise in pixels -> fully core-local, no collectives.

Per core:
  pass A (d=31..0): T-state suffix scan; warp T (ch 0) and src (ch 7..9)
  pass B (d=0..31): over-state forward scan; warp acc (ch 1..3) of plane d+2
  pass C (d=0..31): warp bro = overs[31] (ch 4..6)
Warp of one channel-plane: mm1 (U = Wy @ S, 2 accumulated K-chunks),
ScalarE copy U PSUM->SBUF, PE transpose (2x 128x128), DVE copy U^T
PSUM->SBUF, mm2 (F = (U^T).T @ Wx^T), ScalarE copy, DMA out.

Channel-image SBUF layout: [128, 2*W]; image row r lives at partition
r % 128, columns (r // 128)*W ... +W.  This gives compositing a single
full-width free dim and gives mm1 its two K-chunk row views for free.
"""

import os
import sys

import numpy as np

sys.path.insert(0, "/opt/trn_rl_repo")

P, S, H, W = 32, 4, 256, 256
NCORES = 8
NCH = 10

# Intermediate dtype for the transpose + mm2 chain: "f32" or "bf16".
INTER = os.environ.get("KERNEL_INTER_DT", "f32")


def _compute_sample_coords(mpi_planes, pose_tgt, intrins_src, intrins_tgt):
    """Exact reference math for sample coords, float64. -> ix, iy (P,S,H,W)."""
    Kinv = np.linalg.inv(intrins_tgt.astype(np.float64))
    gx, gy = np.meshgrid(
        np.arange(W, dtype=np.float64), np.arange(H, dtype=np.float64)
    )
    pix = np.stack([gx.ravel(), gy.ravel(), np.ones(H * W)])  # (3, HW)
    cam_dir = Kinv @ pix  # (3, HW)
    ix = np.empty((P, S, H, W))
    iy = np.empty((P, S, H, W))
    for s in range(S):
        K4 = np.zeros((4, 4))
        K4[:3, :3] = intrins_src[s].astype(np.float64)
        K4[3, 3] = 1.0
        proj = K4 @ pose_tgt[s].astype(np.float64)
        for p in range(P):
            cam = np.concatenate(
                [cam_dir * np.float64(mpi_planes[p]), np.ones((1, H * W))], 0
            )
            upc = proj @ cam
            z = upc[2] + 1e-10
            ix[p, s] = (upc[0] / z).reshape(H, W)
            iy[p, s] = (upc[1] / z).reshape(H, W)
    return ix, iy


def _bilinear_matrix(coord_1d, n_in):
    """1D resample matrix M[out, in] with reference tap/validity semantics."""
    n_out = coord_1d.shape[0]
    M = np.zeros((n_out, n_in), np.float64)
    c0 = np.floor(coord_1d)
    w1 = coord_1d - c0
    w0 = 1.0 - w1
    for o in range(n_out):
        i0 = int(c0[o])
        if 0 <= i0 <= n_in - 1:
            M[o, i0] += w0[o]
        if 0 <= i0 + 1 <= n_in - 1:
            M[o, i0 + 1] += w1[o]
    return M


def _reference_numpy(colors, alphas, imgs_src, mpi_planes, pose_tgt,
                     intrins_src, intrins_tgt):
    """Pure-numpy replica of the reference (generic fallback + self-test)."""
    Pn, Sn, Hh, Ww = alphas.shape
    ca = 1.0 - alphas
    pm = colors * alphas[..., None]
    overs = np.empty_like(pm)
    over = np.zeros_like(pm[0])
    for d in range(Pn):
        over = over * ca[d][..., None] + pm[d]
        overs[d] = over
    acc = overs[np.maximum(np.arange(Pn) - 2, 0)]
    bro = np.broadcast_to(overs[-1][None], (Pn, Sn, Hh, Ww, 3))
    rc = np.cumprod(ca[::-1], axis=0)[::-1]
    T = np.concatenate([rc[1:], np.ones_like(rc[:1])], axis=0)
    src = np.broadcast_to(imgs_src[None], (Pn, Sn, Hh, Ww, 3))
    stacked = np.concatenate([T[..., None], acc, bro, src], axis=-1)

    ix, iy = _compute_sample_coords(mpi_planes, pose_tgt, intrins_src,
                                    intrins_tgt)
    out = np.empty((Pn, Sn, NCH, Hh, Ww), np.float32)
    for p in range(Pn):
        for s in range(Sn):
            img = stacked[p, s]
            x0 = np.floor(ix[p, s])
            y0 = np.floor(iy[p, s])
            wx1 = ix[p, s] - x0
            wx0 = 1.0 - wx1
            wy1 = iy[p, s] - y0
            wy0 = 1.0 - wy1

            def gather(xx, yy):
                valid = (xx >= 0) & (xx <= Ww - 1) & (yy >= 0) & (yy <= Hh - 1)
                xc = np.clip(xx, 0, Ww - 1).astype(np.int64)
                yc = np.clip(yy, 0, Hh - 1).astype(np.int64)
                return img[yc, xc] * valid[..., None]

            warped = (gather(x0, y0) * (wx0 * wy0)[..., None]
                      + gather(x0 + 1, y0) * (wx1 * wy0)[..., None]
                      + gather(x0, y0 + 1) * (wx0 * wy1)[..., None]
                      + gather(x0 + 1, y0 + 1) * (wx1 * wy1)[..., None])
            out[p, s] = warped.transpose(2, 0, 1).astype(np.float32)
    return out


_CACHED = {}


def _build_bass_program():
    """Build (once) the SPMD Bass program shared by all 8 cores."""
    if "nc" in _CACHED:
        return _CACHED["nc"]

    import concourse.bacc as bacc
    import concourse.mybir as mybir
    from concourse import tile

    f32 = mybir.dt.float32
    f32r = mybir.dt.float32r
    bf16 = mybir.dt.bfloat16
    inter_dt = bf16 if INTER == "bf16" else f32r

    nc = bacc.Bacc(
        "TRN2", target_bir_lowering=False, debug=False,
        enable_asserts=False, num_devices=NCORES,
    )

    alphas_d = nc.dram_tensor("alphas", [P, H, W], f32, kind="ExternalInput").ap()
    colors_d = nc.dram_tensor("colors", [P, 3, H, W], f32, kind="ExternalInput").ap()
    src_d = nc.dram_tensor("src", [3, H, W], f32r, kind="ExternalInput").ap()
    wyt_d = nc.dram_tensor("wyt", [P, H, 128], f32r, kind="ExternalInput").ap()
    wxt_d = nc.dram_tensor("wxt", [P, 2, 128, W], inter_dt,
                           kind="ExternalInput").ap()
    ident_d = nc.dram_tensor("ident", [128, 128], inter_dt,
                             kind="ExternalInput").ap()
    zeros_d = nc.dram_tensor("zeros", [128, 2 * W], f32r,
                             kind="ExternalInput").ap()
    ones_d = nc.dram_tensor("ones", [128, 2 * W], f32r,
                            kind="ExternalInput").ap()
    out_d = nc.dram_tensor("out", [P, NCH, 128, W], f32,
                           kind="ExternalOutput").ap()

    with tile.TileContext(nc) as tc:
        with (
            tc.tile_pool(name="alpha", bufs=P) as alpha_pool,
            tc.tile_pool(name="persist", bufs=1) as persist,
            tc.tile_pool(name="wy", bufs=P) as wy_pool,
            tc.tile_pool(name="wx", bufs=4) as wx_pool,
            tc.tile_pool(name="cols", bufs=3) as cols_pool,
            tc.tile_pool(name="work", bufs=5) as work,
            tc.tile_pool(name="tmp", bufs=2) as tmpp,
            tc.tile_pool(name="psum", bufs=2, space="PSUM") as psum,
        ):
            ident_sb = persist.tile([128, 128], inter_dt, tag="ident", name="ident_sb")
            nc.sync.dma_start(ident_sb[:], ident_d[:])

            over_sb = [persist.tile([128, 2 * W], f32r, tag=f"over{c}", name=f"over_sb{c}")
                       for c in range(3)]
            t_sb = persist.tile([128, 2 * W], f32r, tag="tchan", name="t_sb")
            src_sb = [persist.tile([128, 2 * W], f32r, tag=f"src{c}", name=f"src_sb{c}")
                      for c in range(3)]
            for c in range(3):
                nc.sync.dma_start(over_sb[c][:], zeros_d[:])
                nc.sync.dma_start(
                    src_sb[c][:].rearrange("p (c w) -> p c w", c=2),
                    src_d[c].rearrange("(c p) w -> p c w", p=128),
                )
            nc.sync.dma_start(t_sb[:], ones_d[:])

            alpha_sb = [alpha_pool.tile([128, 2 * W], f32, tag="alpha", name="alpha_sb")
                        for _ in range(P)]
            wy_sb = [wy_pool.tile([128, 256], f32r, tag="wy", name="wy_sb") for _ in range(P)]
            for d in range(P):
                nc.sync.dma_start(
                    wy_sb[d][:].rearrange("p (c m) -> p c m", c=2),
                    wyt_d[d].rearrange("(c p) m -> p c m", p=128),
                )

            def load_wx(d):
                wx_t = wx_pool.tile([128, 2 * W], inter_dt, tag="wx", name="wx_t")
                nc.sync.dma_start(
                    wx_t[:].rearrange("p (c m) -> p c m", c=2),
                    wxt_d[d].rearrange("c p m -> p c m"),
                )
                return wx_t

            def warp(d, s_tile, ci, wx_t):
                """Warp channel image s_tile with plane-d matrices -> out[d,ci]."""
                U = psum.tile([128, W], f32, tag="U", name="U_ps")
                nc.tensor.matmul(
                    U[:], wy_sb[d][:, 0:128], s_tile[:, 0:W],
                    start=True, stop=False,
                )
                nc.tensor.matmul(
                    U[:], wy_sb[d][:, 128:256], s_tile[:, W:2 * W],
                    start=False, stop=True,
                )
                u_sb = work.tile([128, W], inter_dt, tag="u_sb", name="u_sb")
                nc.scalar.copy(u_sb[:], U[:])
                UT = psum.tile([128, W], inter_dt, tag="UT", name="UT_ps")
                nc.tensor.transpose(UT[:, 0:128], u_sb[:, 0:128], ident_sb[:])
                nc.tensor.transpose(UT[:, 128:256], u_sb[:, 128:256],
                                    ident_sb[:])
                ut_sb = work.tile([128, W], inter_dt, tag="ut_sb", name="ut_sb")
                nc.vector.tensor_copy(ut_sb[:], UT[:])
                F = psum.tile([128, W], f32, tag="F", name="F_ps")
                nc.tensor.matmul(F[:], ut_sb[:, 0:128], wx_t[:, 0:W],
                                 start=True, stop=False)
                nc.tensor.matmul(F[:], ut_sb[:, 128:256], wx_t[:, W:2 * W],
                                 start=False, stop=True)
                f_sb = work.tile([128, W], f32, tag="f_sb", name="f_sb")
                nc.scalar.copy(f_sb[:], F[:])
                nc.sync.dma_start(out_d[d, ci], f_sb[:])

            # ---- pass A: backward suffix scan for T; warp T + src ------
            for d in range(P - 1, -1, -1):
                nc.sync.dma_start(
                    alpha_sb[d][:].rearrange("p (c w) -> p c w", c=2),
                    alphas_d[d].rearrange("(c p) w -> p c w", p=128),
                )
                wx_t = load_wx(d)
                warp(d, t_sb, 0, wx_t)
                for c in range(3):
                    warp(d, src_sb[c], 7 + c, wx_t)
                # T <- T * (1 - alpha_d)   (ordered after the T warp's reads)
                tt = tmpp.tile([128, 2 * W], f32, tag="t_tmp", name="t_tmp")
                nc.gpsimd.tensor_mul(tt[:], t_sb[:], alpha_sb[d][:])
                nc.gpsimd.tensor_sub(t_sb[:], t_sb[:], tt[:])

            # ---- pass B: forward over scan; warp acc -------------------
            for d in range(P):
                col_t = [cols_pool.tile([128, 2 * W], f32, tag=f"col{c}", name=f"col_t{c}")
                         for c in range(3)]
                for c in range(3):
                    nc.sync.dma_start(
                        col_t[c][:].rearrange("p (c2 w) -> p c2 w", c2=2),
                        colors_d[d, c].rearrange("(c2 p) w -> p c2 w", p=128),
                    )
                # over_c += alpha_d * (colors_c - over_c)
                for c in range(3):
                    eng = nc.vector if c < 2 else nc.gpsimd
                    t = tmpp.tile([128, 2 * W], f32, tag=f"ov_tmp{c}", name=f"ov_tmp{c}")
                    eng.tensor_sub(t[:], col_t[c][:], over_sb[c][:])
                    eng.tensor_mul(t[:], t[:], alpha_sb[d][:])
                    eng.tensor_add(over_sb[c][:], over_sb[c][:], t[:])
                # over == overs[d]; acc[pl] = overs[max(pl-2, 0)]
                if d == 0:
                    for pl in (0, 1, 2):
                        wx_t = load_wx(pl)
                        for c in range(3):
                            warp(pl, over_sb[c], 1 + c, wx_t)
                elif d <= P - 3:
                    wx_t = load_wx(d + 2)
                    for c in range(3):
                        warp(d + 2, over_sb[c], 1 + c, wx_t)

            # ---- pass C: warp bro = overs[-1] --------------------------
            for d in range(P):
                wx_t = load_wx(d)
                for c in range(3):
                    warp(d, over_sb[c], 4 + c, wx_t)

    nc.compile()
    _CACHED["nc"] = nc
    return nc


def _host_prepare(colors, alphas, imgs_src, mpi_planes, pose_tgt,
                  intrins_src, intrins_tgt):
    """Build per-core input maps. Returns (in_maps, separable)."""
    import ml_dtypes

    ix, iy = _compute_sample_coords(mpi_planes, pose_tgt, intrins_src,
                                    intrins_tgt)
    dev_x = np.abs(ix - ix[:, :, :1, :]).max()
    dev_y = np.abs(iy - iy[:, :, :, :1]).max()
    if dev_x > 1e-3 or dev_y > 1e-3:
        return None, False

    ix1 = ix[:, :, 0, :]  # (P, S, W)
    iy1 = iy[:, :, :, 0]  # (P, S, H)

    inter_np = ml_dtypes.bfloat16 if INTER == "bf16" else np.float32
    ident = np.eye(128, dtype=np.float32).astype(inter_np)

    in_maps = []
    for core in range(NCORES):
        s, h = divmod(core, 2)
        wyt = np.zeros((P, H, 128), np.float32)
        wxt = np.zeros((P, W, W), np.float32)
        for d in range(P):
            My = _bilinear_matrix(iy1[d, s, h * 128:(h + 1) * 128], H)
            wyt[d] = My.T.astype(np.float32)  # [yi, yo]
            Mx = _bilinear_matrix(ix1[d, s], W)
            wxt[d] = Mx.T.astype(np.float32)  # [xi, xo]
        in_maps.append({
            "alphas": np.ascontiguousarray(alphas[:, s]),
            "colors": np.ascontiguousarray(colors[:, s].transpose(0, 3, 1, 2)),
            "src": np.ascontiguousarray(imgs_src[s].transpose(2, 0, 1)),
            "wyt": wyt,
            "wxt": np.ascontiguousarray(
                wxt.reshape(P, 2, 128, W)).astype(inter_np),
            "ident": ident,
            "zeros": np.zeros((128, 2 * W), np.float32),
            "ones": np.ones((128, 2 * W), np.float32),
        })
    return in_maps, True


def kernel(colors, alphas, imgs_src, mpi_planes, pose_tgt, intrins_src,
           intrins_tgt):
    colors = np.asarray(colors, np.float32)
    alphas = np.asarray(alphas, np.float32)
    imgs_src = np.asarray(imgs_src, np.float32)
    mpi_planes = np.asarray(mpi_planes, np.float32)
    pose_tgt = np.asarray(pose_tgt, np.float32)
    intrins_src = np.asarray(intrins_src, np.float32)
    intrins_tgt = np.asarray(intrins_tgt, np.float32)

    in_maps, separable = _host_prepare(
        colors, alphas, imgs_src, mpi_planes, pose_tgt, intrins_src,
        intrins_tgt)
    if not separable:
        return _reference_numpy(colors, alphas, imgs_src, mpi_planes,
                                pose_tgt, intrins_src, intrins_tgt)

    from concourse.bass_utils import run_bass_kernel_spmd

    nc = _build_bass_program()
    _CACHED["last_in_maps"] = in_maps
    res = run_bass_kernel_spmd(nc, in_maps, core_ids=list(range(NCORES)))
    _CACHED["last_results"] = res

    out = np.empty((P, S, NCH, H, W), np.float32)
    for core in range(NCORES):
        s, h = divmod(core, 2)
        out[:, s, :, h * 128:(h + 1) * 128, :] = res.results[core]["out"]
    return out

